# revision 13
# baseline (speedup 1.0000x reference)
"""BiDAF-style co-attention (memory_len=2) Trainium2 Bass kernel.

Full inputs:
  input     [8, 4096, 1024] f32
  memory    [8, 2, 1024]    f32
  w_input   [1024] f32, w_memory [1024] f32, dot_scale [1024] f32
Output:
  concat([input, output_one, input*output_one, output_two*output_one], -1)
  -> [8, 4096, 4096] f32

Sharding: data-parallel over batch; core b gets batch b (8 cores).

Math (per batch):
  v_m   = w_input + dot_scale * mem_m            (d-vector, m=0,1)
  c_m   = mem_m . w_memory                       (scalar)
  att[l,m] = input[l] . v_m + c_m                (fused mult-reduce DVE ops
                                                  on bf16 inputs, f32 accum)
  amax[l] = max_m att[l,m]  (shifted by -c0; softmax over L is shift-inv)
  e_m[l] = exp(att[l,m]-amax[l]); r[l] = 1/(e0+e1)
  output_one[l] = r[l] * (e0[l]*mem0 + e1[l]*mem1)
                  (PE bf16 rank-2 matmul, lhsT=est pair, rhs=mem rows;
                   ACT applies the r scale on the PSUM->SBUF copy)
  comp2[l] = input[l]*output_one[l]              (DVE f32)
  wexp[l] = exp(amax[l]); o2raw = sum_l wexp[l]*input[l]
            (PE bf16 matvec, PSUM-accumulated across all 32 row tiles)
  comp3[l] = output_two*output_one = (r[l]/stot)*(e0*q0raw + e1*q1raw),
             q_mraw = o2raw*mem_m  (PE bf16 rank-2 with est stationaries;
                                    r/stot applied on the ACT copy)

Performance structure (the kernel is HBM-write-bound, ~330-420 GB/s):
  - compute is sized to finish by ~110 us while the 67 MiB of output
    writes stream continuously to ~210 us; SBUF-resident input is the
    write backlog for the sync queue.
  - sync  (SP HWDGE):  per group: 8 tile loads (prefetched one group
    ahead) then ONE batched 4 MiB comp0 write; tail: every other 1 MiB
    comp3 pair write.
  - scalar(ACT HWDGE): combined o1+comp2 [128, 2D] writes as produced;
    tail: the other comp3 pair writes.
  - comp3 is produced in pairs of row tiles so tail DMAs are 1 MiB.
All broadcasts are built on-chip via PE (no DRAM round trips).
"""

import numpy as np

B, L, D = 8, 4096, 1024
T = L // 128  # 32 row-tiles of 128
G = 8         # tiles per group (batched small ops)
DEFER = 16    # last tiles whose o1+comp2 run during the comp3 tail

_CACHE = {}

# stats column layout ([128, NSTAT] f32), blocks of 32 (col t = tile t)
A0 = 0      # att0
A1 = 32     # att1
AM = 64     # amax
E0 = 96     # e0arg -> e0   (E1 = E0+32 so (e0_t, e1_t) is a stride-32 pair)
E1 = 128    # e1arg -> e1
SS = 160    # e0+e1
RR = 192    # r = 1/(e0+e1)
RS = 224    # r / stot  (comp3 copy scale)
WE = 256    # wexp = exp(amax), f32 (o2 matvec stationaries)
MD, CF, SE, ST, SRB, CD = 288, 289, 290, 291, 292, 293
MROW = 294  # 294,295: [c0, c1] staging on partition 0
NSTAT = 296


def _build():
    import concourse.bacc as bacc
    import concourse.bass as bass
    import concourse.tile as tile
    from concourse import mybir
    from concourse.masks import make_identity

    f32 = mybir.dt.float32
    bf16 = mybir.dt.bfloat16
    ALU = mybir.AluOpType
    ACT = mybir.ActivationFunctionType

    nc = bacc.Bacc("TRN2", target_bir_lowering=False, debug=False)

    inp = nc.dram_tensor("input", [L, D], f32, kind="ExternalInput").ap()
    mem = nc.dram_tensor("memory", [2, D], f32, kind="ExternalInput").ap()
    w_in = nc.dram_tensor("w_input", [D], f32, kind="ExternalInput").ap()
    w_mem = nc.dram_tensor("w_memory", [D], f32, kind="ExternalInput").ap()
    d_sc = nc.dram_tensor("dot_scale", [D], f32, kind="ExternalInput").ap()
    out = nc.dram_tensor("out", [L, 4 * D], f32, kind="ExternalOutput").ap()

    def bc(src_ap, n_part, n_free):
        # broadcast-read AP: n_part partitions each reading the same n_free
        # contiguous elements at src_ap's offset (DMA-only pattern)
        return bass.AP(src_ap.tensor, src_ap.offset, [[0, n_part], [1, n_free]])

    def comp0_group_ap(g):
        # DRAM AP for out[g*G*128:(g+1)*G*128, 0:D] as [p, t, d]: element
        # offset = ((g*G+t)*128+p)*4096 + d
        return bass.AP(
            out.tensor,
            g * G * 128 * 4 * D,
            [[4 * D, 128], [128 * 4 * D, G], [1, D]],
        )

    def comp3_pair_ap(i):
        # DRAM AP for out[i*256:(i+1)*256, 3D:4D] as [p, t, d]
        return bass.AP(
            out.tensor,
            (i * 256) * 4 * D + 3 * D,
            [[4 * D, 128], [128 * 4 * D, 2], [1, D]],
        )

    ts = bass.ts

    with tile.TileContext(nc) as tc:
        with (
            tc.tile_pool(name="consts", bufs=1) as consts,
            tc.tile_pool(name="inbf", bufs=2) as inbfp,
            tc.tile_pool(name="stage12", bufs=3) as stage12p,
            tc.tile_pool(name="stage3", bufs=3) as stage3p,
            tc.tile_pool(name="pbig", bufs=2, space="PSUM") as pbig,
            tc.tile_pool(name="psmall", bufs=2, space="PSUM") as psmall,
            tc.tile_pool(name="o2ps", bufs=1, space="PSUM") as o2psp,
        ):
            # ---------------- constants / setup ----------------
            # all 32 input row-tiles stay resident (128 KiB/partition);
            # this is also the sync queue's write backlog for comp0
            inp_all = consts.tile([128, T * D], f32)
            mem_sb = consts.tile([2, D], f32)
            nc.scalar.dma_start(out=mem_sb, in_=mem)
            stats = consts.tile([128, NSTAT], f32)
            identity = consts.tile([128, 128], f32)
            make_identity(nc, identity)
            ones_row = consts.tile([1, 128], f32)
            nc.vector.memset(ones_row, 1.0)
            # est: per-tile transposed [e0;e1] stationaries (bf16), col t
            est = consts.tile([2, T * 128], bf16)
            mem_bf = consts.tile([2, D], bf16)
            nc.scalar.copy(mem_bf, mem_sb)
            qraw_bf = consts.tile([2, D], bf16)
            ones12_bf = consts.tile([1, 2], bf16)
            nc.vector.memset(ones12_bf, 1.0)
            garb_bf = consts.tile([128, D], bf16)  # att-dot discard target
            # last group's o2 matvec runs in bf16 (f32 MMs at HAM half
            # clock would delay finalize); webf8 = its bf16 wexp columns
            webf8 = consts.tile([128, G], bf16)
            v0b = consts.tile([128, D], bf16)
            v1b = consts.tile([128, D], bf16)

            # first group of loads up front (prefetch)
            for t in range(0, G):
                nc.sync.dma_start(
                    out=inp_all[:, ts(t, D)], in_=inp[ts(t, 128), :]
                )

            # v_m = w_in + ds*mem_m computed on partition-0 rows, then
            # PE-broadcast to 128 partitions (no DRAM round trip):
            # out[p, n] = ones_row[0, p] * v_row[0, n]
            ds_row = stage12p.tile([1, D], f32, tag="s12")
            nc.scalar.dma_start(out=ds_row, in_=bc(d_sc, 1, D))
            win_row = stage12p.tile([1, D], f32, tag="s12")
            nc.scalar.dma_start(out=win_row, in_=bc(w_in, 1, D))
            m0_row = stage3p.tile([1, D], f32, tag="s3")
            nc.scalar.dma_start(out=m0_row, in_=bc(mem[0, :], 1, D))
            m1_row = stage3p.tile([1, D], f32, tag="s3")
            nc.scalar.dma_start(out=m1_row, in_=bc(mem[1, :], 1, D))
            vrows = stage12p.tile([1, 2 * D], f32, tag="s12")
            nc.vector.scalar_tensor_tensor(
                out=vrows[:, 0:D], in0=m0_row, scalar=1.0, in1=ds_row,
                op0=ALU.mult, op1=ALU.mult,
            )
            nc.vector.tensor_tensor(
                out=vrows[:, 0:D], in0=vrows[:, 0:D], in1=win_row, op=ALU.add
            )
            nc.vector.tensor_tensor(
                out=vrows[:, D : 2 * D], in0=m1_row, in1=ds_row, op=ALU.mult
            )
            nc.vector.tensor_tensor(
                out=vrows[:, D : 2 * D],
                in0=vrows[:, D : 2 * D],
                in1=win_row,
                op=ALU.add,
            )
            for vi, dst in enumerate((v0b, v1b)):
                vps = pbig.tile([128, D], f32, tag="pb")
                for h in range(2):
                    nc.tensor.matmul(
                        vps[:, ts(h, 512)],
                        lhsT=ones_row,
                        rhs=vrows[:, vi * D + h * 512 : vi * D + (h + 1) * 512],
                        start=True,
                        stop=True,
                    )
                nc.scalar.copy(dst, vps)

            # memdot = (mem * w_memory).sum(-1) -> [2,1]; cdiff = c1-c0
            # broadcast to all partitions fully on-chip via PE
            wmem_b = stage3p.tile([2, D], f32, tag="s3")
            nc.scalar.dma_start(out=wmem_b, in_=bc(w_mem, 2, D))
            sc2 = stage3p.tile([2, D], f32, tag="s3")
            nc.vector.scalar_tensor_tensor(
                out=sc2, in0=mem_sb, scalar=1.0, in1=wmem_b,
                op0=ALU.mult, op1=ALU.mult,
                accum_out=stats[0:2, MD : MD + 1],
            )
            md_ps = psmall.tile([1, 2], f32, tag="ps")
            nc.tensor.transpose(
                md_ps, stats[0:2, MD : MD + 1], identity[0:2, 0:2]
            )
            md_row = stats[0:1, MROW : MROW + 2]  # [c0, c1] on partition 0
            nc.scalar.copy(md_row, md_ps)
            nc.vector.tensor_tensor(
                out=stats[0:1, CF : CF + 1],
                in0=md_row[:, 1:2],
                in1=md_row[:, 0:1],
                op=ALU.subtract,
            )
            cd_ps = psmall.tile([128, 1], f32, tag="ps")
            nc.tensor.matmul(
                cd_ps, lhsT=ones_row, rhs=stats[0:1, CF : CF + 1],
                start=True, stop=True,
            )
            nc.scalar.copy(stats[:, CD : CD + 1], cd_ps)

            cdc = stats[:, CD : CD + 1]

            # output_two numerator accumulator (PSUM, lives across the loop)
            o2ps = o2psp.tile([1, D], f32, tag="o2")

            # strided pair view: pair_view[:, t, :] = cols (E0+t, E1+t)
            pair_view = stats[:, E0 : E0 + 64].rearrange("p (a b) -> p b a", a=2)

            def emit_st12(t, queue):
                # output_one = r * (e0*mem0 + e1*mem1): PE bf16 rank-2 matmul
                # with the est pair stationary; ACT applies the r scale on
                # the PSUM->SBUF copy; comp2 = input*output_one on DVE (f32)
                o1_ps = pbig.tile([128, D], f32, tag="pb")
                for h in range(2):
                    nc.tensor.matmul(
                        o1_ps[:, ts(h, 512)],
                        lhsT=est[:, ts(t, 128)],
                        rhs=mem_bf[:, ts(h, 512)],
                        start=True,
                        stop=True,
                    )
                st12 = stage12p.tile([128, 2 * D], f32, tag="s12")
                nc.scalar.activation(
                    out=st12[:, 0:D], in_=o1_ps, func=ACT.Copy,
                    scale=stats[:, RR + t : RR + t + 1],
                )
                nc.vector.tensor_tensor(
                    out=st12[:, D : 2 * D],
                    in0=inp_all[:, ts(t, D)], in1=st12[:, 0:D],
                    op=ALU.mult,
                )
                queue.dma_start(out=out[ts(t, 128), D : 3 * D], in_=st12)

            def emit_finalize():
                # stot = sum_l wexp: row-reduce, transpose, reduce, PE bcast
                nc.vector.tensor_reduce(
                    out=stats[:, SE : SE + 1], in_=stats[:, WE : WE + T],
                    axis=mybir.AxisListType.X, op=ALU.add,
                )
                se_ps = psmall.tile([1, 128], f32, tag="ps")
                nc.tensor.transpose(se_ps, stats[:, SE : SE + 1], identity)
                se_row = stage3p.tile([1, 128], f32, tag="s3")
                nc.scalar.copy(se_row, se_ps)
                nc.vector.tensor_reduce(
                    out=stats[0:1, ST : ST + 1], in_=se_row,
                    axis=mybir.AxisListType.X, op=ALU.add,
                )
                nc.vector.reciprocal(
                    stats[0:1, CF : CF + 1], stats[0:1, ST : ST + 1]
                )
                srb_ps = psmall.tile([128, 1], f32, tag="ps")
                nc.tensor.matmul(
                    srb_ps, lhsT=ones_row, rhs=stats[0:1, CF : CF + 1],
                    start=True, stop=True,
                )
                nc.scalar.copy(stats[:, SRB : SRB + 1], srb_ps)
                # rs = r / stot (per-row comp3 scale)
                nc.vector.tensor_scalar_mul(
                    stats[:, RS : RS + T],
                    stats[:, RR : RR + T],
                    stats[:, SRB : SRB + 1],
                )
                # qraw = o2raw (bcast 1->2 partitions via PE) * mem
                o2row = stage3p.tile([1, D], f32, tag="s3")
                nc.scalar.copy(o2row, o2ps)
                o2row_bf = stage3p.tile([1, D], bf16, tag="s3")
                nc.scalar.copy(o2row_bf, o2row)
                q2_ps = pbig.tile([2, D], f32, tag="pb")
                for h in range(2):
                    nc.tensor.matmul(
                        q2_ps[:, ts(h, 512)],
                        lhsT=ones12_bf,
                        rhs=o2row_bf[:, ts(h, 512)],
                        start=True,
                        stop=True,
                    )
                nc.vector.tensor_tensor(
                    out=qraw_bf, in0=q2_ps, in1=mem_sb, op=ALU.mult
                )

            # ---------------- main pass -----------------------------------
            for g in range(0, T // G):
                # prefetch next group's loads; then the batched comp0 write
                # of the previous group (keeps the sync queue streaming)
                if g + 1 < T // G:
                    for t in range((g + 1) * G, (g + 2) * G):
                        nc.sync.dma_start(
                            out=inp_all[:, ts(t, D)], in_=inp[ts(t, 128), :]
                        )
                if g > 0:
                    nc.sync.dma_start(
                        out=comp0_group_ap(g - 1),
                        in_=inp_all[:, (g - 1) * G * D : g * G * D],
                    )
                # per-tile: bf16 cast (ACT), att dots (DVE bf16, f32 accum),
                # amax + wexp, o2p matvec (PE bf16) immediately
                for t in range(g * G, (g + 1) * G):
                    in_t = inp_all[:, ts(t, D)]
                    nc.vector.scalar_tensor_tensor(
                        out=garb_bf, in0=in_t, scalar=1.0, in1=v0b,
                        op0=ALU.mult, op1=ALU.mult,
                        accum_out=stats[:, A0 + t : A0 + t + 1],
                    )
                    nc.vector.scalar_tensor_tensor(
                        out=garb_bf, in0=in_t, scalar=1.0, in1=v1b,
                        op0=ALU.mult, op1=ALU.mult,
                        accum_out=stats[:, A1 + t : A1 + t + 1],
                    )
                    # amax = max(a1 + cdiff, a0); wexp = exp(amax) -> bf16
                    nc.vector.scalar_tensor_tensor(
                        out=stats[:, AM + t : AM + t + 1],
                        in0=stats[:, A1 + t : A1 + t + 1], scalar=cdc,
                        in1=stats[:, A0 + t : A0 + t + 1],
                        op0=ALU.add, op1=ALU.max,
                    )
                    nc.scalar.activation(
                        out=stats[:, WE + t : WE + t + 1],
                        in_=stats[:, AM + t : AM + t + 1], func=ACT.Exp,
                    )
                    # output_two partials on PE: o2ps += wexp_t^T @ in_t.
                    # Plain f32 matmuls (no cast needed) except the last
                    # group, which goes bf16 so the PE stream reaches the
                    # o2 stop + finalize quickly at loop end.
                    if t < T - G:
                        for h in range(2):
                            nc.tensor.matmul(
                                o2ps[:, ts(h, 512)],
                                lhsT=stats[:, WE + t : WE + t + 1],
                                rhs=in_t[:, ts(h, 512)],
                                start=(t == 0),
                                stop=False,
                            )
                    else:
                        tl = t - (T - G)
                        nc.scalar.activation(
                            out=webf8[:, tl : tl + 1],
                            in_=stats[:, AM + t : AM + t + 1], func=ACT.Exp,
                        )
                        in_bf = inbfp.tile([128, D], bf16, tag="ibf")
                        nc.scalar.copy(in_bf, in_t)
                        for h in range(2):
                            nc.tensor.matmul(
                                o2ps[:, ts(h, 512)],
                                lhsT=webf8[:, tl : tl + 1],
                                rhs=in_bf[:, ts(h, 512)],
                                start=False,
                                stop=(t == T - 1),
                            )

                # batched group stats ([128, G] blocks)
                gc = g * G
                a0b = stats[:, A0 + gc : A0 + gc + G]
                a1b = stats[:, A1 + gc : A1 + gc + G]
                amb = stats[:, AM + gc : AM + gc + G]
                e0b = stats[:, E0 + gc : E0 + gc + G]
                e1b = stats[:, E1 + gc : E1 + gc + G]
                ssb = stats[:, SS + gc : SS + gc + G]
                rrb = stats[:, RR + gc : RR + gc + G]
                # e0arg = a0 - amax ; e1arg = (a1 + cdiff) - amax
                nc.vector.tensor_tensor(out=e0b, in0=a0b, in1=amb, op=ALU.subtract)
                nc.vector.scalar_tensor_tensor(
                    out=e1b, in0=a1b, scalar=cdc, in1=amb,
                    op0=ALU.add, op1=ALU.subtract,
                )
                nc.scalar.activation(out=e0b, in_=e0b, func=ACT.Exp)
                nc.scalar.activation(out=e1b, in_=e1b, func=ACT.Exp)
                nc.vector.tensor_tensor(out=ssb, in0=e0b, in1=e1b, op=ALU.add)
                nc.vector.reciprocal(rrb, ssb)

                # per-tile: est stationaries (PE transpose + ACT copy), then
                # o1 (PE+ACT) and comp2 (DVE); combined [128, 2D] write on
                # the scalar queue
                for t in range(gc, gc + G):
                    wst_ps = psmall.tile([2, 128], f32, tag="ps")
                    nc.tensor.transpose(wst_ps, pair_view[:, t, :], identity)
                    nc.scalar.copy(est[:, ts(t, 128)], wst_ps)
                    if t < T - DEFER:
                        emit_st12(t, nc.scalar)

            # last group's comp0 write
            nc.sync.dma_start(
                out=comp0_group_ap(T // G - 1),
                in_=inp_all[:, (T - G) * D : T * D],
            )

            # deferred st12 tiles first: they have no finalize dependency,
            # so their PE/ACT/DVE work and writes bridge the loop->tail
            # transition while finalize's o2row round-trip completes
            for i in range(DEFER):
                emit_st12(
                    T - DEFER + i, nc.sync if i % 2 == 0 else nc.scalar
                )

            # finalize feeds the comp3 tail (all on-chip)
            emit_finalize()

            # ---------------- comp3 tail (PE + ACT) -----------------------
            # produced in pairs of row tiles -> 1 MiB writes, queues
            # alternating per pair
            for i in range(T // 2):
                st3 = stage3p.tile([128, 2 * D], f32, tag="s3")
                for j in range(2):
                    tt = 2 * i + j
                    o3_ps = pbig.tile([128, D], f32, tag="pb")
                    for h in range(2):
                        nc.tensor.matmul(
                            o3_ps[:, ts(h, 512)],
                            lhsT=est[:, ts(tt, 128)],
                            rhs=qraw_bf[:, ts(h, 512)],
                            start=True,
                            stop=True,
                        )
                    if j == 0:
                        nc.scalar.activation(
                            out=st3[:, ts(j, D)], in_=o3_ps, func=ACT.Copy,
                            scale=stats[:, RS + tt : RS + tt + 1],
                        )
                    else:
                        nc.vector.tensor_scalar_mul(
                            st3[:, ts(j, D)], o3_ps,
                            stats[:, RS + tt : RS + tt + 1],
                        )
                q = nc.scalar if i % 2 == 0 else nc.sync
                q.dma_start(out=comp3_pair_ap(i), in_=st3)

    nc.compile()
    return nc


def _get_nc():
    if "nc" not in _CACHE:
        _CACHE["nc"] = _build()
    return _CACHE["nc"]


def kernel(input, memory, w_input, w_memory, dot_scale):
    from concourse.bass_utils import run_bass_kernel_spmd

    nc = _get_nc()
    input = np.ascontiguousarray(input, dtype=np.float32)
    memory = np.ascontiguousarray(memory, dtype=np.float32)
    w_input = np.ascontiguousarray(w_input, dtype=np.float32)
    w_memory = np.ascontiguousarray(w_memory, dtype=np.float32)
    dot_scale = np.ascontiguousarray(dot_scale, dtype=np.float32)
    in_maps = [
        {
            "input": input[b],
            "memory": memory[b],
            "w_input": w_input,
            "w_memory": w_memory,
            "dot_scale": dot_scale,
        }
        for b in range(B)
    ]
    res = run_bass_kernel_spmd(nc, in_maps, core_ids=list(range(B)))
    return np.stack([res.results[b]["out"] for b in range(B)], axis=0)


# revision 14
# speedup vs baseline: 1.1316x; 1.1316x over previous
"""BiDAF-style co-attention (memory_len=2) Trainium2 Bass kernel.

Full inputs:
  input     [8, 4096, 1024] f32
  memory    [8, 2, 1024]    f32
  w_input   [1024] f32, w_memory [1024] f32, dot_scale [1024] f32
Output:
  concat([input, output_one, input*output_one, output_two*output_one], -1)
  -> [8, 4096, 4096] f32

Sharding: data-parallel over batch; core b gets batch b (8 cores).

Math (per batch):
  v_m   = w_input + dot_scale * mem_m            (d-vector, m=0,1)
  c_m   = mem_m . w_memory                       (scalar)
  att[l,m] = input[l] . v_m + c_m                (fused mult-reduce DVE ops
                                                  on bf16 inputs, f32 accum)
  amax[l] = max_m att[l,m]  (shifted by -c0; softmax over L is shift-inv)
  e_m[l] = exp(att[l,m]-amax[l]); r[l] = 1/(e0+e1)
  output_one[l] = r[l] * (e0[l]*mem0 + e1[l]*mem1)
                  (PE bf16 rank-2 matmul, lhsT=est pair, rhs=mem rows;
                   ACT applies the r scale on the PSUM->SBUF copy)
  comp2[l] = input[l]*output_one[l]              (DVE f32)
  wexp[l] = exp(amax[l]); o2raw = sum_l wexp[l]*input[l]
            (PE bf16 matvec, PSUM-accumulated across all 32 row tiles)
  comp3[l] = output_two*output_one = (r[l]/stot)*(e0*q0raw + e1*q1raw),
             q_mraw = o2raw*mem_m  (PE bf16 rank-2 with est stationaries;
                                    r/stot applied on the ACT copy)

Performance structure (the kernel is HBM-write-bound, ~330-420 GB/s):
  - compute is sized to finish by ~110 us while the 67 MiB of output
    writes stream continuously to ~210 us; SBUF-resident input is the
    write backlog for the sync queue.
  - sync  (SP HWDGE):  per group: 8 tile loads (prefetched one group
    ahead) then ONE batched 4 MiB comp0 write; tail: every other 1 MiB
    comp3 pair write.
  - scalar(ACT HWDGE): combined o1+comp2 [128, 2D] writes as produced;
    tail: the other comp3 pair writes.
  - comp3 is produced in pairs of row tiles so tail DMAs are 1 MiB.
All broadcasts are built on-chip via PE (no DRAM round trips).
"""

import numpy as np

B, L, D = 8, 4096, 1024
T = L // 128  # 32 row-tiles of 128
G = 8         # tiles per group (batched small ops)
DEFER = 16    # last tiles whose o1+comp2 run during the comp3 tail

_CACHE = {}

# stats column layout ([128, NSTAT] f32), blocks of 32 (col t = tile t)
A0 = 0      # att0
A1 = 32     # att1
AM = 64     # amax
E0 = 96     # e0arg -> e0   (E1 = E0+32 so (e0_t, e1_t) is a stride-32 pair)
E1 = 128    # e1arg -> e1
SS = 160    # e0+e1
RR = 192    # r = 1/(e0+e1)
RS = 224    # r / stot  (comp3 copy scale)
WE = 256    # wexp = exp(amax), f32 (o2 matvec stationaries)
MD, CF, SE, ST, SRB, CD = 288, 289, 290, 291, 292, 293
MROW = 294  # 294,295: [c0, c1] staging on partition 0
NSTAT = 296


def _build():
    import concourse.bacc as bacc
    import concourse.bass as bass
    import concourse.tile as tile
    from concourse import mybir
    from concourse.masks import make_identity

    f32 = mybir.dt.float32
    bf16 = mybir.dt.bfloat16
    ALU = mybir.AluOpType
    ACT = mybir.ActivationFunctionType

    nc = bacc.Bacc("TRN2", target_bir_lowering=False, debug=False)

    inp = nc.dram_tensor("input", [L, D], f32, kind="ExternalInput").ap()
    mem = nc.dram_tensor("memory", [2, D], f32, kind="ExternalInput").ap()
    w_in = nc.dram_tensor("w_input", [D], f32, kind="ExternalInput").ap()
    w_mem = nc.dram_tensor("w_memory", [D], f32, kind="ExternalInput").ap()
    d_sc = nc.dram_tensor("dot_scale", [D], f32, kind="ExternalInput").ap()
    out = nc.dram_tensor("out", [L, 4 * D], f32, kind="ExternalOutput").ap()

    def bc(src_ap, n_part, n_free):
        # broadcast-read AP: n_part partitions each reading the same n_free
        # contiguous elements at src_ap's offset (DMA-only pattern)
        return bass.AP(src_ap.tensor, src_ap.offset, [[0, n_part], [1, n_free]])

    def comp0_group_ap(g):
        # DRAM AP for out[g*G*128:(g+1)*G*128, 0:D] as [p, t, d]: element
        # offset = ((g*G+t)*128+p)*4096 + d
        return bass.AP(
            out.tensor,
            g * G * 128 * 4 * D,
            [[4 * D, 128], [128 * 4 * D, G], [1, D]],
        )

    def comp3_pair_ap(i):
        # DRAM AP for out[i*256:(i+1)*256, 3D:4D] as [p, t, d]
        return bass.AP(
            out.tensor,
            (i * 256) * 4 * D + 3 * D,
            [[4 * D, 128], [128 * 4 * D, 2], [1, D]],
        )

    ts = bass.ts

    with tile.TileContext(nc) as tc:
        with (
            tc.tile_pool(name="consts", bufs=1) as consts,
            tc.tile_pool(name="stage12", bufs=3) as stage12p,
            tc.tile_pool(name="stage3", bufs=3) as stage3p,
            tc.tile_pool(name="pbig", bufs=2, space="PSUM") as pbig,
            tc.tile_pool(name="psmall", bufs=2, space="PSUM") as psmall,
            tc.tile_pool(name="o2ps", bufs=1, space="PSUM") as o2psp,
        ):
            # ---------------- constants / setup ----------------
            # all 32 input row-tiles stay resident (128 KiB/partition);
            # this is also the sync queue's write backlog for comp0
            inp_all = consts.tile([128, T * D], f32)
            mem_sb = consts.tile([2, D], f32)
            nc.scalar.dma_start(out=mem_sb, in_=mem)
            stats = consts.tile([128, NSTAT], f32)
            identity = consts.tile([128, 128], f32)
            make_identity(nc, identity)
            ones_row = consts.tile([1, 128], f32)
            nc.vector.memset(ones_row, 1.0)
            # est: per-tile transposed [e0;e1] stationaries (bf16), col t
            est = consts.tile([2, T * 128], bf16)
            mem_bf = consts.tile([2, D], bf16)
            nc.scalar.copy(mem_bf, mem_sb)
            qraw_bf = consts.tile([2, D], bf16)
            ones12_bf = consts.tile([1, 2], bf16)
            nc.vector.memset(ones12_bf, 1.0)
            garb_bf = consts.tile([128, D], bf16)  # att-dot discard target
            v0b = consts.tile([128, D], bf16)
            v1b = consts.tile([128, D], bf16)

            # first group of loads up front (prefetch)
            for t in range(0, G):
                nc.sync.dma_start(
                    out=inp_all[:, ts(t, D)], in_=inp[ts(t, 128), :]
                )

            # v_m = w_in + ds*mem_m computed on partition-0 rows, then
            # PE-broadcast to 128 partitions (no DRAM round trip):
            # out[p, n] = ones_row[0, p] * v_row[0, n]
            ds_row = stage12p.tile([1, D], f32, tag="s12")
            nc.scalar.dma_start(out=ds_row, in_=bc(d_sc, 1, D))
            win_row = stage12p.tile([1, D], f32, tag="s12")
            nc.scalar.dma_start(out=win_row, in_=bc(w_in, 1, D))
            m0_row = stage3p.tile([1, D], f32, tag="s3")
            nc.scalar.dma_start(out=m0_row, in_=bc(mem[0, :], 1, D))
            m1_row = stage3p.tile([1, D], f32, tag="s3")
            nc.scalar.dma_start(out=m1_row, in_=bc(mem[1, :], 1, D))
            vrows = stage12p.tile([1, 2 * D], f32, tag="s12")
            nc.vector.scalar_tensor_tensor(
                out=vrows[:, 0:D], in0=m0_row, scalar=1.0, in1=ds_row,
                op0=ALU.mult, op1=ALU.mult,
            )
            nc.vector.tensor_tensor(
                out=vrows[:, 0:D], in0=vrows[:, 0:D], in1=win_row, op=ALU.add
            )
            nc.vector.tensor_tensor(
                out=vrows[:, D : 2 * D], in0=m1_row, in1=ds_row, op=ALU.mult
            )
            nc.vector.tensor_tensor(
                out=vrows[:, D : 2 * D],
                in0=vrows[:, D : 2 * D],
                in1=win_row,
                op=ALU.add,
            )
            for vi, dst in enumerate((v0b, v1b)):
                vps = pbig.tile([128, D], f32, tag="pb")
                for h in range(2):
                    nc.tensor.matmul(
                        vps[:, ts(h, 512)],
                        lhsT=ones_row,
                        rhs=vrows[:, vi * D + h * 512 : vi * D + (h + 1) * 512],
                        start=True,
                        stop=True,
                    )
                nc.scalar.copy(dst, vps)

            # memdot = (mem * w_memory).sum(-1) -> [2,1]; cdiff = c1-c0
            # broadcast to all partitions fully on-chip via PE
            wmem_b = stage3p.tile([2, D], f32, tag="s3")
            nc.scalar.dma_start(out=wmem_b, in_=bc(w_mem, 2, D))
            sc2 = stage3p.tile([2, D], f32, tag="s3")
            nc.vector.scalar_tensor_tensor(
                out=sc2, in0=mem_sb, scalar=1.0, in1=wmem_b,
                op0=ALU.mult, op1=ALU.mult,
                accum_out=stats[0:2, MD : MD + 1],
            )
            md_ps = psmall.tile([1, 2], f32, tag="ps")
            nc.tensor.transpose(
                md_ps, stats[0:2, MD : MD + 1], identity[0:2, 0:2]
            )
            md_row = stats[0:1, MROW : MROW + 2]  # [c0, c1] on partition 0
            nc.scalar.copy(md_row, md_ps)
            nc.vector.tensor_tensor(
                out=stats[0:1, CF : CF + 1],
                in0=md_row[:, 1:2],
                in1=md_row[:, 0:1],
                op=ALU.subtract,
            )
            cd_ps = psmall.tile([128, 1], f32, tag="ps")
            nc.tensor.matmul(
                cd_ps, lhsT=ones_row, rhs=stats[0:1, CF : CF + 1],
                start=True, stop=True,
            )
            nc.scalar.copy(stats[:, CD : CD + 1], cd_ps)

            cdc = stats[:, CD : CD + 1]

            # output_two numerator accumulator (PSUM, lives across the loop)
            o2ps = o2psp.tile([1, D], f32, tag="o2")

            # strided pair view: pair_view[:, t, :] = cols (E0+t, E1+t)
            pair_view = stats[:, E0 : E0 + 64].rearrange("p (a b) -> p b a", a=2)

            def emit_st12(t, queue):
                # output_one = r * (e0*mem0 + e1*mem1): PE bf16 rank-2 matmul
                # with the est pair stationary; ACT applies the r scale on
                # the PSUM->SBUF copy; comp2 = input*output_one on DVE (f32)
                o1_ps = pbig.tile([128, D], f32, tag="pb")
                for h in range(2):
                    nc.tensor.matmul(
                        o1_ps[:, ts(h, 512)],
                        lhsT=est[:, ts(t, 128)],
                        rhs=mem_bf[:, ts(h, 512)],
                        start=True,
                        stop=True,
                    )
                st12 = stage12p.tile([128, 2 * D], f32, tag="s12")
                nc.scalar.activation(
                    out=st12[:, 0:D], in_=o1_ps, func=ACT.Copy,
                    scale=stats[:, RR + t : RR + t + 1],
                )
                nc.vector.tensor_tensor(
                    out=st12[:, D : 2 * D],
                    in0=inp_all[:, ts(t, D)], in1=st12[:, 0:D],
                    op=ALU.mult,
                )
                queue.dma_start(out=out[ts(t, 128), D : 3 * D], in_=st12)

            def emit_finalize():
                # stot = sum_l wexp: row-reduce, transpose, reduce, PE bcast
                nc.vector.tensor_reduce(
                    out=stats[:, SE : SE + 1], in_=stats[:, WE : WE + T],
                    axis=mybir.AxisListType.X, op=ALU.add,
                )
                se_ps = psmall.tile([1, 128], f32, tag="ps")
                nc.tensor.transpose(se_ps, stats[:, SE : SE + 1], identity)
                se_row = stage3p.tile([1, 128], f32, tag="s3")
                nc.scalar.copy(se_row, se_ps)
                nc.vector.tensor_reduce(
                    out=stats[0:1, ST : ST + 1], in_=se_row,
                    axis=mybir.AxisListType.X, op=ALU.add,
                )
                nc.vector.reciprocal(
                    stats[0:1, CF : CF + 1], stats[0:1, ST : ST + 1]
                )
                srb_ps = psmall.tile([128, 1], f32, tag="ps")
                nc.tensor.matmul(
                    srb_ps, lhsT=ones_row, rhs=stats[0:1, CF : CF + 1],
                    start=True, stop=True,
                )
                nc.scalar.copy(stats[:, SRB : SRB + 1], srb_ps)
                # rs = r / stot (per-row comp3 scale)
                nc.vector.tensor_scalar_mul(
                    stats[:, RS : RS + T],
                    stats[:, RR : RR + T],
                    stats[:, SRB : SRB + 1],
                )
                # qraw = o2raw (bcast 1->2 partitions via PE) * mem
                o2row = stage3p.tile([1, D], f32, tag="s3")
                nc.scalar.copy(o2row, o2ps)
                o2row_bf = stage3p.tile([1, D], bf16, tag="s3")
                nc.scalar.copy(o2row_bf, o2row)
                q2_ps = pbig.tile([2, D], f32, tag="pb")
                for h in range(2):
                    nc.tensor.matmul(
                        q2_ps[:, ts(h, 512)],
                        lhsT=ones12_bf,
                        rhs=o2row_bf[:, ts(h, 512)],
                        start=True,
                        stop=True,
                    )
                nc.vector.tensor_tensor(
                    out=qraw_bf, in0=q2_ps, in1=mem_sb, op=ALU.mult
                )

            # ---------------- main pass -----------------------------------
            for g in range(0, T // G):
                # prefetch next group's loads; then the batched comp0 write
                # of the previous group (keeps the sync queue streaming)
                if g + 1 < T // G:
                    for t in range((g + 1) * G, (g + 2) * G):
                        nc.sync.dma_start(
                            out=inp_all[:, ts(t, D)], in_=inp[ts(t, 128), :]
                        )
                if g > 0:
                    nc.sync.dma_start(
                        out=comp0_group_ap(g - 1),
                        in_=inp_all[:, (g - 1) * G * D : g * G * D],
                    )
                # per-tile: bf16 cast (ACT), att dots (DVE bf16, f32 accum),
                # amax + wexp, o2p matvec (PE bf16) immediately
                for t in range(g * G, (g + 1) * G):
                    in_t = inp_all[:, ts(t, D)]
                    nc.vector.scalar_tensor_tensor(
                        out=garb_bf, in0=in_t, scalar=1.0, in1=v0b,
                        op0=ALU.mult, op1=ALU.mult,
                        accum_out=stats[:, A0 + t : A0 + t + 1],
                    )
                    nc.vector.scalar_tensor_tensor(
                        out=garb_bf, in0=in_t, scalar=1.0, in1=v1b,
                        op0=ALU.mult, op1=ALU.mult,
                        accum_out=stats[:, A1 + t : A1 + t + 1],
                    )
                    # amax = max(a1 + cdiff, a0); wexp = exp(amax) -> bf16
                    nc.vector.scalar_tensor_tensor(
                        out=stats[:, AM + t : AM + t + 1],
                        in0=stats[:, A1 + t : A1 + t + 1], scalar=cdc,
                        in1=stats[:, A0 + t : A0 + t + 1],
                        op0=ALU.add, op1=ALU.max,
                    )
                    nc.scalar.activation(
                        out=stats[:, WE + t : WE + t + 1],
                        in_=stats[:, AM + t : AM + t + 1], func=ACT.Exp,
                    )
                    # output_two partials on PE: o2ps += wexp_t^T @ in_t
                    # (plain f32 matmul: 4 cyc/row but no bf16 cast needed)
                    for h in range(2):
                        nc.tensor.matmul(
                            o2ps[:, ts(h, 512)],
                            lhsT=stats[:, WE + t : WE + t + 1],
                            rhs=in_t[:, ts(h, 512)],
                            start=(t == 0),
                            stop=(t == T - 1),
                        )

                # batched group stats ([128, G] blocks)
                gc = g * G
                a0b = stats[:, A0 + gc : A0 + gc + G]
                a1b = stats[:, A1 + gc : A1 + gc + G]
                amb = stats[:, AM + gc : AM + gc + G]
                e0b = stats[:, E0 + gc : E0 + gc + G]
                e1b = stats[:, E1 + gc : E1 + gc + G]
                ssb = stats[:, SS + gc : SS + gc + G]
                rrb = stats[:, RR + gc : RR + gc + G]
                # e0arg = a0 - amax ; e1arg = (a1 + cdiff) - amax
                nc.vector.tensor_tensor(out=e0b, in0=a0b, in1=amb, op=ALU.subtract)
                nc.vector.scalar_tensor_tensor(
                    out=e1b, in0=a1b, scalar=cdc, in1=amb,
                    op0=ALU.add, op1=ALU.subtract,
                )
                nc.scalar.activation(out=e0b, in_=e0b, func=ACT.Exp)
                nc.scalar.activation(out=e1b, in_=e1b, func=ACT.Exp)
                nc.vector.tensor_tensor(out=ssb, in0=e0b, in1=e1b, op=ALU.add)
                nc.vector.reciprocal(rrb, ssb)

                # per-tile: est stationaries (PE transpose + ACT copy), then
                # o1 (PE+ACT) and comp2 (DVE); combined [128, 2D] write on
                # the scalar queue
                for t in range(gc, gc + G):
                    wst_ps = psmall.tile([2, 128], f32, tag="ps")
                    nc.tensor.transpose(wst_ps, pair_view[:, t, :], identity)
                    nc.scalar.copy(est[:, ts(t, 128)], wst_ps)
                    if t < T - DEFER:
                        emit_st12(t, nc.scalar)

            # last group's comp0 write
            nc.sync.dma_start(
                out=comp0_group_ap(T // G - 1),
                in_=inp_all[:, (T - G) * D : T * D],
            )

            # finalize feeds the comp3 tail (all on-chip)
            emit_finalize()

            # ---------------- comp3 tail (PE + ACT) -----------------------
            # deferred st12 tiles interleave with the comp3 pairs so both
            # write streams progress together; pairs -> 1 MiB writes with
            # queues alternating per pair
            for i in range(T // 2):
                if i < DEFER:
                    emit_st12(
                        T - DEFER + i, nc.sync if i % 2 == 0 else nc.scalar
                    )
                st3 = stage3p.tile([128, 2 * D], f32, tag="s3")
                for j in range(2):
                    tt = 2 * i + j
                    o3_ps = pbig.tile([128, D], f32, tag="pb")
                    for h in range(2):
                        nc.tensor.matmul(
                            o3_ps[:, ts(h, 512)],
                            lhsT=est[:, ts(tt, 128)],
                            rhs=qraw_bf[:, ts(h, 512)],
                            start=True,
                            stop=True,
                        )
                    if j == 0:
                        nc.scalar.activation(
                            out=st3[:, ts(j, D)], in_=o3_ps, func=ACT.Copy,
                            scale=stats[:, RS + tt : RS + tt + 1],
                        )
                    else:
                        nc.vector.tensor_scalar_mul(
                            st3[:, ts(j, D)], o3_ps,
                            stats[:, RS + tt : RS + tt + 1],
                        )
                q = nc.scalar if i % 2 == 0 else nc.sync
                q.dma_start(out=comp3_pair_ap(i), in_=st3)

    nc.compile()
    return nc


def _get_nc():
    if "nc" not in _CACHE:
        _CACHE["nc"] = _build()
    return _CACHE["nc"]


def kernel(input, memory, w_input, w_memory, dot_scale):
    from concourse.bass_utils import run_bass_kernel_spmd

    nc = _get_nc()
    input = np.ascontiguousarray(input, dtype=np.float32)
    memory = np.ascontiguousarray(memory, dtype=np.float32)
    w_input = np.ascontiguousarray(w_input, dtype=np.float32)
    w_memory = np.ascontiguousarray(w_memory, dtype=np.float32)
    dot_scale = np.ascontiguousarray(dot_scale, dtype=np.float32)
    in_maps = [
        {
            "input": input[b],
            "memory": memory[b],
            "w_input": w_input,
            "w_memory": w_memory,
            "dot_scale": dot_scale,
        }
        for b in range(B)
    ]
    res = run_bass_kernel_spmd(nc, in_maps, core_ids=list(range(B)))
    return np.stack([res.results[b]["out"] for b in range(B)], axis=0)


# revision 15
# speedup vs baseline: 1.1544x; 1.0201x over previous
"""BiDAF-style co-attention (memory_len=2) Trainium2 Bass kernel.

Full inputs:
  input     [8, 4096, 1024] f32
  memory    [8, 2, 1024]    f32
  w_input   [1024] f32, w_memory [1024] f32, dot_scale [1024] f32
Output:
  concat([input, output_one, input*output_one, output_two*output_one], -1)
  -> [8, 4096, 4096] f32

Sharding: data-parallel over batch; core b gets batch b (8 cores).

Math (per batch):
  v_m   = w_input + dot_scale * mem_m            (d-vector, m=0,1)
  c_m   = mem_m . w_memory                       (scalar)
  att[l,m] = input[l] . v_m + c_m                (fused mult-reduce DVE ops
                                                  on bf16 inputs, f32 accum)
  amax[l] = max_m att[l,m]  (shifted by -c0; softmax over L is shift-inv)
  e_m[l] = exp(att[l,m]-amax[l]); r[l] = 1/(e0+e1)
  output_one[l] = r[l] * (e0[l]*mem0 + e1[l]*mem1)
                  (PE bf16 rank-2 matmul, lhsT=est pair, rhs=mem rows;
                   ACT applies the r scale on the PSUM->SBUF copy)
  comp2[l] = input[l]*output_one[l]              (DVE f32)
  wexp[l] = exp(amax[l]); o2raw = sum_l wexp[l]*input[l]
            (PE bf16 matvec, PSUM-accumulated across all 32 row tiles)
  comp3[l] = output_two*output_one = (r[l]/stot)*(e0*q0raw + e1*q1raw),
             q_mraw = o2raw*mem_m  (PE bf16 rank-2 with est stationaries;
                                    r/stot applied on the ACT copy)

Performance structure (the kernel is HBM-write-bound, ~330-420 GB/s):
  - compute is sized to finish by ~110 us while the 67 MiB of output
    writes stream continuously to ~210 us; SBUF-resident input is the
    write backlog for the sync queue.
  - sync  (SP HWDGE):  per group: 8 tile loads (prefetched one group
    ahead) then ONE batched 4 MiB comp0 write; tail: every other 1 MiB
    comp3 pair write.
  - scalar(ACT HWDGE): combined o1+comp2 [128, 2D] writes as produced;
    tail: the other comp3 pair writes.
  - comp3 is produced in pairs of row tiles so tail DMAs are 1 MiB.
All broadcasts are built on-chip via PE (no DRAM round trips).
"""

import numpy as np

B, L, D = 8, 4096, 1024
T = L // 128  # 32 row-tiles of 128
G = 8         # tiles per group (batched small ops)
DEFER = 8     # last tiles whose o1+comp2 run during the comp3 tail

_CACHE = {}

# stats column layout ([128, NSTAT] f32), blocks of 32 (col t = tile t)
A0 = 0      # att0
A1 = 32     # att1
AM = 64     # amax
E0 = 96     # e0arg -> e0   (E1 = E0+32 so (e0_t, e1_t) is a stride-32 pair)
E1 = 128    # e1arg -> e1
SS = 160    # e0+e1
RR = 192    # r = 1/(e0+e1)
RS = 224    # r / stot  (comp3 copy scale)
WE = 256    # wexp = exp(amax), f32 (o2 matvec stationaries)
MD, CF, SE, ST, SRB, CD = 288, 289, 290, 291, 292, 293
MROW = 294  # 294,295: [c0, c1] staging on partition 0
NSTAT = 296


def _build():
    import concourse.bacc as bacc
    import concourse.bass as bass
    import concourse.tile as tile
    from concourse import mybir
    from concourse.masks import make_identity

    f32 = mybir.dt.float32
    bf16 = mybir.dt.bfloat16
    ALU = mybir.AluOpType
    ACT = mybir.ActivationFunctionType

    nc = bacc.Bacc("TRN2", target_bir_lowering=False, debug=False)

    inp = nc.dram_tensor("input", [L, D], f32, kind="ExternalInput").ap()
    mem = nc.dram_tensor("memory", [2, D], f32, kind="ExternalInput").ap()
    w_in = nc.dram_tensor("w_input", [D], f32, kind="ExternalInput").ap()
    w_mem = nc.dram_tensor("w_memory", [D], f32, kind="ExternalInput").ap()
    d_sc = nc.dram_tensor("dot_scale", [D], f32, kind="ExternalInput").ap()
    out = nc.dram_tensor("out", [L, 4 * D], f32, kind="ExternalOutput").ap()

    def bc(src_ap, n_part, n_free):
        # broadcast-read AP: n_part partitions each reading the same n_free
        # contiguous elements at src_ap's offset (DMA-only pattern)
        return bass.AP(src_ap.tensor, src_ap.offset, [[0, n_part], [1, n_free]])

    def comp0_group_ap(g):
        # DRAM AP for out[g*G*128:(g+1)*G*128, 0:D] as [p, t, d]: element
        # offset = ((g*G+t)*128+p)*4096 + d
        return bass.AP(
            out.tensor,
            g * G * 128 * 4 * D,
            [[4 * D, 128], [128 * 4 * D, G], [1, D]],
        )

    def comp3_pair_ap(i):
        # DRAM AP for out[i*256:(i+1)*256, 3D:4D] as [p, t, d]
        return bass.AP(
            out.tensor,
            (i * 256) * 4 * D + 3 * D,
            [[4 * D, 128], [128 * 4 * D, 2], [1, D]],
        )

    ts = bass.ts

    with tile.TileContext(nc) as tc:
        with (
            tc.tile_pool(name="consts", bufs=1) as consts,
            tc.tile_pool(name="inbf", bufs=3) as inbfp,
            tc.tile_pool(name="stage12", bufs=3) as stage12p,
            tc.tile_pool(name="stage3", bufs=3) as stage3p,
            tc.tile_pool(name="pbig", bufs=2, space="PSUM") as pbig,
            tc.tile_pool(name="psmall", bufs=2, space="PSUM") as psmall,
            tc.tile_pool(name="o2ps", bufs=1, space="PSUM") as o2psp,
        ):
            # ---------------- constants / setup ----------------
            # all 32 input row-tiles stay resident (128 KiB/partition);
            # this is also the sync queue's write backlog for comp0
            inp_all = consts.tile([128, T * D], f32)
            mem_sb = consts.tile([2, D], f32)
            nc.scalar.dma_start(out=mem_sb, in_=mem)
            stats = consts.tile([128, NSTAT], f32)
            identity = consts.tile([128, 128], f32)
            make_identity(nc, identity)
            ones_row = consts.tile([1, 128], f32)
            nc.vector.memset(ones_row, 1.0)
            # est: per-tile transposed [e0;e1] stationaries (bf16), col t
            est = consts.tile([2, T * 128], bf16)
            mem_bf = consts.tile([2, D], bf16)
            nc.scalar.copy(mem_bf, mem_sb)
            qraw_bf = consts.tile([2, D], bf16)
            ones12_bf = consts.tile([1, 2], bf16)
            nc.vector.memset(ones12_bf, 1.0)
            # wexp = exp(amax) -> bf16 cols (o2 matvec stationaries)
            webf = consts.tile([128, T], bf16)
            garb_bf = consts.tile([128, D], bf16)  # att-dot discard target
            v0b = consts.tile([128, D], bf16)
            v1b = consts.tile([128, D], bf16)

            # first group of loads up front (prefetch)
            for t in range(0, G):
                nc.sync.dma_start(
                    out=inp_all[:, ts(t, D)], in_=inp[ts(t, 128), :]
                )

            # v_m = w_in + ds*mem_m computed on partition-0 rows, then
            # PE-broadcast to 128 partitions (no DRAM round trip):
            # out[p, n] = ones_row[0, p] * v_row[0, n]
            ds_row = stage12p.tile([1, D], f32, tag="s12")
            nc.scalar.dma_start(out=ds_row, in_=bc(d_sc, 1, D))
            win_row = stage12p.tile([1, D], f32, tag="s12")
            nc.scalar.dma_start(out=win_row, in_=bc(w_in, 1, D))
            m0_row = stage3p.tile([1, D], f32, tag="s3")
            nc.scalar.dma_start(out=m0_row, in_=bc(mem[0, :], 1, D))
            m1_row = stage3p.tile([1, D], f32, tag="s3")
            nc.scalar.dma_start(out=m1_row, in_=bc(mem[1, :], 1, D))
            vrows = stage12p.tile([1, 2 * D], f32, tag="s12")
            nc.vector.scalar_tensor_tensor(
                out=vrows[:, 0:D], in0=m0_row, scalar=1.0, in1=ds_row,
                op0=ALU.mult, op1=ALU.mult,
            )
            nc.vector.tensor_tensor(
                out=vrows[:, 0:D], in0=vrows[:, 0:D], in1=win_row, op=ALU.add
            )
            nc.vector.tensor_tensor(
                out=vrows[:, D : 2 * D], in0=m1_row, in1=ds_row, op=ALU.mult
            )
            nc.vector.tensor_tensor(
                out=vrows[:, D : 2 * D],
                in0=vrows[:, D : 2 * D],
                in1=win_row,
                op=ALU.add,
            )
            for vi, dst in enumerate((v0b, v1b)):
                vps = pbig.tile([128, D], f32, tag="pb")
                for h in range(2):
                    nc.tensor.matmul(
                        vps[:, ts(h, 512)],
                        lhsT=ones_row,
                        rhs=vrows[:, vi * D + h * 512 : vi * D + (h + 1) * 512],
                        start=True,
                        stop=True,
                    )
                nc.scalar.copy(dst, vps)

            # memdot = (mem * w_memory).sum(-1) -> [2,1]; cdiff = c1-c0
            # broadcast to all partitions fully on-chip via PE
            wmem_b = stage3p.tile([2, D], f32, tag="s3")
            nc.scalar.dma_start(out=wmem_b, in_=bc(w_mem, 2, D))
            sc2 = stage3p.tile([2, D], f32, tag="s3")
            nc.vector.scalar_tensor_tensor(
                out=sc2, in0=mem_sb, scalar=1.0, in1=wmem_b,
                op0=ALU.mult, op1=ALU.mult,
                accum_out=stats[0:2, MD : MD + 1],
            )
            md_ps = psmall.tile([1, 2], f32, tag="ps")
            nc.tensor.transpose(
                md_ps, stats[0:2, MD : MD + 1], identity[0:2, 0:2]
            )
            md_row = stats[0:1, MROW : MROW + 2]  # [c0, c1] on partition 0
            nc.scalar.copy(md_row, md_ps)
            nc.vector.tensor_tensor(
                out=stats[0:1, CF : CF + 1],
                in0=md_row[:, 1:2],
                in1=md_row[:, 0:1],
                op=ALU.subtract,
            )
            cd_ps = psmall.tile([128, 1], f32, tag="ps")
            nc.tensor.matmul(
                cd_ps, lhsT=ones_row, rhs=stats[0:1, CF : CF + 1],
                start=True, stop=True,
            )
            nc.scalar.copy(stats[:, CD : CD + 1], cd_ps)

            cdc = stats[:, CD : CD + 1]

            # output_two numerator accumulator (PSUM, lives across the loop)
            o2ps = o2psp.tile([1, D], f32, tag="o2")

            # strided pair view: pair_view[:, t, :] = cols (E0+t, E1+t)
            pair_view = stats[:, E0 : E0 + 64].rearrange("p (a b) -> p b a", a=2)

            def emit_st12(t, queue):
                # output_one = r * (e0*mem0 + e1*mem1): PE bf16 rank-2 matmul
                # with the est pair stationary; ACT applies the r scale on
                # the PSUM->SBUF copy; comp2 = input*output_one on DVE (f32)
                o1_ps = pbig.tile([128, D], f32, tag="pb")
                for h in range(2):
                    nc.tensor.matmul(
                        o1_ps[:, ts(h, 512)],
                        lhsT=est[:, ts(t, 128)],
                        rhs=mem_bf[:, ts(h, 512)],
                        start=True,
                        stop=True,
                    )
                st12 = stage12p.tile([128, 2 * D], f32, tag="s12")
                nc.scalar.activation(
                    out=st12[:, 0:D], in_=o1_ps, func=ACT.Copy,
                    scale=stats[:, RR + t : RR + t + 1],
                )
                nc.vector.tensor_tensor(
                    out=st12[:, D : 2 * D],
                    in0=inp_all[:, ts(t, D)], in1=st12[:, 0:D],
                    op=ALU.mult,
                )
                queue.dma_start(out=out[ts(t, 128), D : 3 * D], in_=st12)

            def emit_finalize():
                # stot = sum_l wexp: row-reduce, transpose, reduce, PE bcast
                nc.vector.tensor_reduce(
                    out=stats[:, SE : SE + 1], in_=webf,
                    axis=mybir.AxisListType.X, op=ALU.add,
                )
                se_ps = psmall.tile([1, 128], f32, tag="ps")
                nc.tensor.transpose(se_ps, stats[:, SE : SE + 1], identity)
                se_row = stage3p.tile([1, 128], f32, tag="s3")
                nc.scalar.copy(se_row, se_ps)
                nc.vector.tensor_reduce(
                    out=stats[0:1, ST : ST + 1], in_=se_row,
                    axis=mybir.AxisListType.X, op=ALU.add,
                )
                nc.vector.reciprocal(
                    stats[0:1, CF : CF + 1], stats[0:1, ST : ST + 1]
                )
                srb_ps = psmall.tile([128, 1], f32, tag="ps")
                nc.tensor.matmul(
                    srb_ps, lhsT=ones_row, rhs=stats[0:1, CF : CF + 1],
                    start=True, stop=True,
                )
                nc.scalar.copy(stats[:, SRB : SRB + 1], srb_ps)
                # rs = r / stot (per-row comp3 scale)
                nc.vector.tensor_scalar_mul(
                    stats[:, RS : RS + T],
                    stats[:, RR : RR + T],
                    stats[:, SRB : SRB + 1],
                )
                # qraw = o2raw (bcast 1->2 partitions via PE) * mem
                o2row = stage3p.tile([1, D], f32, tag="s3")
                nc.scalar.copy(o2row, o2ps)
                o2row_bf = stage3p.tile([1, D], bf16, tag="s3")
                nc.scalar.copy(o2row_bf, o2row)
                q2_ps = pbig.tile([2, D], f32, tag="pb")
                for h in range(2):
                    nc.tensor.matmul(
                        q2_ps[:, ts(h, 512)],
                        lhsT=ones12_bf,
                        rhs=o2row_bf[:, ts(h, 512)],
                        start=True,
                        stop=True,
                    )
                nc.vector.tensor_tensor(
                    out=qraw_bf, in0=q2_ps, in1=mem_sb, op=ALU.mult
                )

            # ---------------- main pass -----------------------------------
            for g in range(0, T // G):
                # prefetch next group's loads; then the batched comp0 write
                # of the previous group (keeps the sync queue streaming)
                if g + 1 < T // G:
                    for t in range((g + 1) * G, (g + 2) * G):
                        nc.sync.dma_start(
                            out=inp_all[:, ts(t, D)], in_=inp[ts(t, 128), :]
                        )
                if g > 0:
                    nc.sync.dma_start(
                        out=comp0_group_ap(g - 1),
                        in_=inp_all[:, (g - 1) * G * D : g * G * D],
                    )
                # per-tile: bf16 cast (ACT), att dots (DVE bf16, f32 accum),
                # amax + wexp, o2p matvec (PE bf16) immediately
                for t in range(g * G, (g + 1) * G):
                    in_t = inp_all[:, ts(t, D)]
                    # bf16 shadow of the tile for the o2 matvec (ACT has
                    # slack; the cast is off the DVE critical path)
                    in_bf = inbfp.tile([128, D], bf16, tag="ibf")
                    nc.scalar.copy(in_bf, in_t)
                    nc.vector.scalar_tensor_tensor(
                        out=garb_bf, in0=in_t, scalar=1.0, in1=v0b,
                        op0=ALU.mult, op1=ALU.mult,
                        accum_out=stats[:, A0 + t : A0 + t + 1],
                    )
                    nc.vector.scalar_tensor_tensor(
                        out=garb_bf, in0=in_t, scalar=1.0, in1=v1b,
                        op0=ALU.mult, op1=ALU.mult,
                        accum_out=stats[:, A1 + t : A1 + t + 1],
                    )
                    # amax = max(a1 + cdiff, a0); wexp = exp(amax) -> bf16
                    nc.vector.scalar_tensor_tensor(
                        out=stats[:, AM + t : AM + t + 1],
                        in0=stats[:, A1 + t : A1 + t + 1], scalar=cdc,
                        in1=stats[:, A0 + t : A0 + t + 1],
                        op0=ALU.add, op1=ALU.max,
                    )
                    nc.scalar.activation(
                        out=webf[:, t : t + 1],
                        in_=stats[:, AM + t : AM + t + 1], func=ACT.Exp,
                    )
                    # output_two partials on PE: o2ps += wexp_t^T @ in_bf
                    # (bf16: keeps PE fast so no matvec backlog builds up
                    # ahead of the finalize/tail chain)
                    for h in range(2):
                        nc.tensor.matmul(
                            o2ps[:, ts(h, 512)],
                            lhsT=webf[:, t : t + 1],
                            rhs=in_bf[:, ts(h, 512)],
                            start=(t == 0),
                            stop=(t == T - 1),
                        )

                # batched group stats ([128, G] blocks)
                gc = g * G
                a0b = stats[:, A0 + gc : A0 + gc + G]
                a1b = stats[:, A1 + gc : A1 + gc + G]
                amb = stats[:, AM + gc : AM + gc + G]
                e0b = stats[:, E0 + gc : E0 + gc + G]
                e1b = stats[:, E1 + gc : E1 + gc + G]
                ssb = stats[:, SS + gc : SS + gc + G]
                rrb = stats[:, RR + gc : RR + gc + G]
                # e0arg = a0 - amax ; e1arg = (a1 + cdiff) - amax
                nc.vector.tensor_tensor(out=e0b, in0=a0b, in1=amb, op=ALU.subtract)
                nc.vector.scalar_tensor_tensor(
                    out=e1b, in0=a1b, scalar=cdc, in1=amb,
                    op0=ALU.add, op1=ALU.subtract,
                )
                nc.scalar.activation(out=e0b, in_=e0b, func=ACT.Exp)
                nc.scalar.activation(out=e1b, in_=e1b, func=ACT.Exp)
                nc.vector.tensor_tensor(out=ssb, in0=e0b, in1=e1b, op=ALU.add)
                nc.vector.reciprocal(rrb, ssb)

                # per-tile: est stationaries (PE transpose + ACT copy)
                for t in range(gc, gc + G):
                    wst_ps = psmall.tile([2, 128], f32, tag="ps")
                    nc.tensor.transpose(wst_ps, pair_view[:, t, :], identity)
                    nc.scalar.copy(est[:, ts(t, 128)], wst_ps)
                # st12 (o1 PE+ACT, comp2 DVE) for the PREVIOUS group: the
                # one-group lag makes these writes drain exactly when the
                # loop's comp0/load traffic tapers off
                if g > 0:
                    for t in range(gc - G, gc):
                        emit_st12(t, nc.scalar)

            # last group's comp0 write
            nc.sync.dma_start(
                out=comp0_group_ap(T // G - 1),
                in_=inp_all[:, (T - G) * D : T * D],
            )

            # finalize feeds the comp3 tail (all on-chip)
            emit_finalize()

            # ---------------- comp3 tail (PE + ACT) -----------------------
            # deferred st12 tiles interleave with the comp3 pairs so both
            # write streams progress together; pairs -> 1 MiB writes with
            # queues alternating per pair
            for i in range(T // 2):
                if i % 2 == 0 and i // 2 < DEFER:
                    emit_st12(
                        T - DEFER + i // 2,
                        nc.sync if i % 4 == 0 else nc.scalar,
                    )
                st3 = stage3p.tile([128, 2 * D], f32, tag="s3")
                for j in range(2):
                    tt = 2 * i + j
                    o3_ps = pbig.tile([128, D], f32, tag="pb")
                    for h in range(2):
                        nc.tensor.matmul(
                            o3_ps[:, ts(h, 512)],
                            lhsT=est[:, ts(tt, 128)],
                            rhs=qraw_bf[:, ts(h, 512)],
                            start=True,
                            stop=True,
                        )
                    if j == 0:
                        nc.scalar.activation(
                            out=st3[:, ts(j, D)], in_=o3_ps, func=ACT.Copy,
                            scale=stats[:, RS + tt : RS + tt + 1],
                        )
                    else:
                        nc.vector.tensor_scalar_mul(
                            st3[:, ts(j, D)], o3_ps,
                            stats[:, RS + tt : RS + tt + 1],
                        )
                q = nc.scalar if i % 2 == 0 else nc.sync
                q.dma_start(out=comp3_pair_ap(i), in_=st3)

    nc.compile()
    return nc


def _get_nc():
    if "nc" not in _CACHE:
        _CACHE["nc"] = _build()
    return _CACHE["nc"]


def kernel(input, memory, w_input, w_memory, dot_scale):
    from concourse.bass_utils import run_bass_kernel_spmd

    nc = _get_nc()
    input = np.ascontiguousarray(input, dtype=np.float32)
    memory = np.ascontiguousarray(memory, dtype=np.float32)
    w_input = np.ascontiguousarray(w_input, dtype=np.float32)
    w_memory = np.ascontiguousarray(w_memory, dtype=np.float32)
    dot_scale = np.ascontiguousarray(dot_scale, dtype=np.float32)
    in_maps = [
        {
            "input": input[b],
            "memory": memory[b],
            "w_input": w_input,
            "w_memory": w_memory,
            "dot_scale": dot_scale,
        }
        for b in range(B)
    ]
    res = run_bass_kernel_spmd(nc, in_maps, core_ids=list(range(B)))
    return np.stack([res.results[b]["out"] for b in range(B)], axis=0)
